# revision 17
# baseline (speedup 1.0000x reference)
"""Trainium2 Bass kernel for nn_NetCrossing (smoothed segment-crossing count).

Math restructure (vs the reference's per-pair s1..s4 formulation):
  For net with pins q_0..q_{P-1} and chain segments i (q_i -> q_{i+1}):
    G[i,p] = cross(d_i, q_p - q_i)   (= d1x_i*y_p - d1y_i*x_p - c1_i)
    s1(i,j)*s2(i,j) = G[i,j]*G[i,j+1] =: Q[i,j]
    s3(i,j)*s4(i,j) = Q[j,i]
  so with R = sigmoid(MU - Q):
    total = 0.5 * sum_{|i-j|>1, valid, same-side, masked} R[i,j]*R[j,i]
  The side weight w=(1+s_i*s_j)/2 in {0,1} and the |i-j|<=1 exclusion are
  folded into an additive pre-sigmoid kill bias: Q3 = Q - KU with
  KU = U - KILL, U = (128*s_i)*(128*s_j) = +/-16384 and KILL in {BIG, 2*BIG};
  kept cells get Q3 == Q (exactly, up to f32 rounding at 2^14), excluded
  cells get Q3 >= ~16k so the sigmoid is exactly 0.

Sharding: nets are grouped by degree class (degree pattern tiles as
[2,3,4,5,6,8,10,12]; deg 2/3 nets have no non-adjacent segment pairs and are
dropped, masked nets are dropped) and distributed round-robin over 8 cores.
Per (core, class) buckets are padded to a fixed capacity with "kill" nets whose
pins sit on a huge convex polygon (every non-adjacent Q is hugely positive so
every sigmoid is exactly 0).

Engine split per class: GpSimd does segment precompute + the side/adjacency
kill tensor (and the full G/Q pipeline for the small classes); DVE does the
G/Q pipeline for the big classes and the fused product+reduce; ACT evaluates
the sigmoid twice (natural and transposed layout) so the product is a pure
unit-stride DVE op.
"""

import math
import numpy as np

import concourse.bass as bass
import concourse.bacc as bacc
import concourse.mybir as mybir
from concourse import tile
from concourse.bass_utils import run_bass_kernel_spmd

F32 = mybir.dt.float32

MU = 0.01
LAMBDA = 1.0
# processing order: big classes first (DVE) so GpSimd's small classes overlap
CLASSES = [8, 10, 12, 4, 5, 6]
BIG_CLASSES = {8, 10, 12}       # G/Q pipeline on DVE; small classes on GpSimd
NCORES = 8
BIG = 16384.0
SSCALE = 128.0                  # sqrt(BIG); side values become +/-128
R0 = 1000.0                     # kill-polygon radius


def _kill_pattern(S):
    i = np.arange(S)
    k = np.full((S, S), BIG, np.float32)
    k[np.abs(i[:, None] - i[None, :]) <= 1] = 2.0 * BIG
    return k.reshape(-1)


def _pad_polygon(P):
    th = 2.0 * np.pi * np.arange(P) / P
    return (R0 * np.cos(th)).astype(np.float32), (R0 * np.sin(th)).astype(np.float32)


def _layout(npps):
    """Column layout: per class [px|py|sp] block, then the killc block."""
    cls_cols = []
    for P, npp in zip(CLASSES, npps):
        S = P - 1
        cls_cols.append(npp * (2 * P + S))
    kill_cols = [(P - 1) * (P - 1) for P in CLASSES]
    return cls_cols, kill_cols, sum(cls_cols) + sum(kill_cols)


def build_blobs(pos, flat_netpin, netpin_start, net_mask, pin_side):
    """Host-side shard/pack: FULL inputs -> per-core input blobs [128, COLS].

    Returns (blobs, npps) where npps[i] is the nets-per-partition for class i.
    """
    pos = np.asarray(pos)
    flat_netpin = np.asarray(flat_netpin).astype(np.int64)
    netpin_start = np.asarray(netpin_start).astype(np.int64)
    net_mask = np.asarray(net_mask).astype(bool)
    pin_side = np.asarray(pin_side)

    Ptot = pos.shape[0] // 2
    x = pos[:Ptot].astype(np.float32)
    y = pos[Ptot:].astype(np.float32)
    sidev = (2.0 * pin_side.astype(np.float32) - 1.0) * SSCALE

    deg = np.diff(netpin_start)

    per_class = []
    npps = []
    for P in CLASSES:
        S = P - 1
        nets = np.nonzero(net_mask & (deg == P))[0]
        starts = netpin_start[nets]
        pidx = starts[:, None] + np.arange(P)[None, :]
        pins = flat_netpin[pidx]
        pxc = x[pins]
        pyc = y[pins]
        spc = sidev[pins[:, :S]]
        per_class.append((pxc, pyc, spc))
        worst = -(-len(nets) // NCORES)            # max nets on any core
        npps.append(max(1, -(-worst // 128)))

    cls_cols, kill_cols, COLS = _layout(npps)
    blobs = [np.empty((128, COLS), np.float32) for _ in range(NCORES)]

    col = 0
    for ci, P in enumerate(CLASSES):
        S = P - 1
        npp = npps[ci]
        cap = 128 * npp
        pxc, pyc, spc = per_class[ci]
        padx, pady = _pad_polygon(P)
        for core in range(NCORES):
            mpx = pxc[core::NCORES]
            mpy = pyc[core::NCORES]
            msp = spc[core::NCORES]
            m = mpx.shape[0]
            if m > cap:
                raise RuntimeError(
                    f"class deg={P} core={core}: {m} nets exceeds capacity {cap}"
                )
            bx = np.broadcast_to(padx, (cap, P)).copy()
            by = np.broadcast_to(pady, (cap, P)).copy()
            bs = np.full((cap, S), SSCALE, np.float32)
            bx[:m] = mpx
            by[:m] = mpy
            bs[:m] = msp
            b = blobs[core]
            c = col
            b[:, c:c + npp * P] = bx.reshape(128, npp * P)
            c += npp * P
            b[:, c:c + npp * P] = by.reshape(128, npp * P)
            c += npp * P
            b[:, c:c + npp * S] = bs.reshape(128, npp * S)
        col += cls_cols[ci]

    kcol = sum(cls_cols)
    for ci, P in enumerate(CLASSES):
        S = P - 1
        pat = _kill_pattern(S)
        for core in range(NCORES):
            blobs[core][:, kcol:kcol + S * S] = pat[None, :]
        kcol += S * S

    return blobs, npps


def _emit_program(npps):
    """Build the Bass/Tile program (shared by all 8 cores, SPMD)."""
    cls_cols, kill_cols, COLS = _layout(npps)

    nc = bacc.Bacc()
    blob = nc.declare_dram_parameter("blob", [128, COLS], F32, isOutput=False)
    outp = nc.declare_dram_parameter("out", [128, 1], F32, isOutput=True)

    AX = mybir.AxisListType
    OP = mybir.AluOpType
    ACTF = mybir.ActivationFunctionType

    with tile.TileContext(nc) as tc:
        with (
            tc.tile_pool(name="io", bufs=1) as io,
            tc.tile_pool(name="work", bufs=2) as work,
        ):
            # per-class input tiles (separate tiles -> precise DMA deps)
            kcol0 = sum(cls_cols)
            cls_in = []
            col = 0
            for ci, P in enumerate(CLASSES):
                t = io.tile([128, cls_cols[ci]], F32, tag=f"in_{ci}")
                nc.sync.dma_start(t[:], blob[:, col:col + cls_cols[ci]])
                col += cls_cols[ci]
                cls_in.append(t)
            killt = io.tile([128, sum(kill_cols)], F32)
            nc.sync.dma_start(killt[:], blob[:, kcol0:kcol0 + sum(kill_cols)])

            acc = io.tile([128, len(CLASSES)], F32)
            mu_t = io.tile([128, 1], F32)
            nc.vector.memset(mu_t[:], MU)

            kcol = 0
            for ci, P in enumerate(CLASSES):
                S = P - 1
                npp = npps[ci]
                big = P in BIG_CLASSES
                eng = nc.vector if big else nc.gpsimd

                sb = cls_in[ci][:]
                c = 0
                px = sb[:, c:c + npp * P].rearrange("p (n q) -> p n q", n=npp)
                c += npp * P
                py = sb[:, c:c + npp * P].rearrange("p (n q) -> p n q", n=npp)
                c += npp * P
                sp = sb[:, c:c + npp * S].rearrange("p (n s) -> p n s", n=npp)
                kc = killt[:, kcol:kcol + S * S].rearrange("p (a b) -> p a b", a=S)
                kcol += S * S

                def t3(name, n2):
                    t = work.tile([128, npp * n2], F32, tag=name)
                    return t[:].rearrange("p (n q) -> p n q", n=npp)

                def t4(name, a, b):
                    t = work.tile([128, npp * a * b], F32, tag=name)
                    return t[:].rearrange("p (n i j) -> p n i j", n=npp, i=a)

                d1x = t3("d1x", S)
                d1y = t3("d1y", S)
                c1a = t3("c1a", S)
                c1b = t3("c1b", S)
                c1 = t3("c1", S)

                # segment precompute — GpSimd for every class (cheap, off DVE)
                nc.gpsimd.tensor_sub(d1x, px[:, :, 1:P], px[:, :, 0:S])
                nc.gpsimd.tensor_sub(d1y, py[:, :, 1:P], py[:, :, 0:S])
                nc.gpsimd.tensor_mul(c1a, d1x, py[:, :, 0:S])
                nc.gpsimd.tensor_mul(c1b, d1y, px[:, :, 0:S])
                nc.gpsimd.tensor_sub(c1, c1a, c1b)

                # side/adjacency kill tensor KU = s_i*s_j - KILL — GpSimd
                shc = [128, npp, S, S]
                uu4 = t4("uu4", S, S)
                ku4 = t4("ku4", S, S)
                nc.gpsimd.tensor_mul(
                    uu4, sp.unsqueeze(3).broadcast_to(shc),
                    sp.unsqueeze(2).broadcast_to(shc),
                )
                nc.gpsimd.tensor_sub(ku4, uu4, kc.unsqueeze(1).broadcast_to(shc))

                # G stage: G[i,p] = d1x_i*y_p - d1y_i*x_p - c1_i
                sh4 = [128, npp, S, P]
                t1 = t4("t1", S, P)
                t2 = t4("t2", S, P)
                u4 = t4("u4", S, P)
                g4 = t4("g4", S, P)
                eng.tensor_mul(
                    t1, d1x.unsqueeze(3).broadcast_to(sh4),
                    py.unsqueeze(2).broadcast_to(sh4),
                )
                eng.tensor_mul(
                    t2, d1y.unsqueeze(3).broadcast_to(sh4),
                    px.unsqueeze(2).broadcast_to(sh4),
                )
                eng.tensor_sub(u4, t1, t2)
                eng.tensor_sub(g4, u4, c1.unsqueeze(3).broadcast_to(sh4))

                # Q = G[:, :, i, j] * G[:, :, i, j+1];  Q3 = Q - KU
                q4 = t4("q4", S, S)
                q3 = t4("q3", S, S)
                eng.tensor_mul(q4, g4[:, :, :, 0:S], g4[:, :, :, 1:P])
                eng.tensor_sub(q3, q4, ku4)

                # R = sigmoid(MU - Q3), emitted twice: natural + transposed
                # (distinct slots per class: the ACT write must carry only one
                # HW wait, so it cannot afford a WAR wait from slot reuse)
                r4 = t4(f"r4_{ci}", S, S)
                rt4 = t4(f"rt4_{ci}", S, S)
                q3f = q3.rearrange("p n i j -> p (n i j)")
                nc.scalar.activation(
                    r4.rearrange("p n i j -> p (n i j)"), q3f,
                    ACTF.Sigmoid, bias=mu_t[:], scale=-1.0,
                )
                nc.scalar.activation(
                    rt4.transpose([0, 1, 3, 2]), q3,
                    ACTF.Sigmoid, bias=mu_t[:], scale=-1.0,
                )

                # fused product + reduction: acc[:, ci] = sum(r * r^T)
                ts4 = t4("ts4", S, S)
                nc.vector.scalar_tensor_tensor(
                    out=ts4.rearrange("p n i j -> p (n i j)"),
                    in0=r4.rearrange("p n i j -> p (n i j)"),
                    scalar=0.0,
                    in1=rt4.rearrange("p n i j -> p (n i j)"),
                    op0=OP.bypass,
                    op1=OP.mult,
                    accum_out=acc[:, ci:ci + 1],
                )

            accfin = io.tile([128, 1], F32)
            nc.vector.tensor_reduce(accfin[:], acc[:], AX.X, OP.add)
            nc.sync.dma_start(outp[:], accfin[:])

    # bacc legalization (splits multi-sem waits: HW allows 1 wait/instruction)
    nc.compile()
    return nc


def run_on_hw(blobs, npps, trace=False, **kw):
    nc = _emit_program(npps)
    in_maps = [{"blob": blobs[c]} for c in range(NCORES)]
    br = run_bass_kernel_spmd(nc, in_maps, list(range(NCORES)), trace=trace, **kw)
    total = 0.0
    for c in range(NCORES):
        total += float(np.asarray(br.results[c]["out"], np.float64).sum())
    total *= 0.5 * LAMBDA
    return np.float32(total), br


def kernel(pos, flat_netpin, netpin_start, net_mask, pin_side):
    blobs, npps = build_blobs(pos, flat_netpin, netpin_start, net_mask, pin_side)
    total, _ = run_on_hw(blobs, npps, trace=False)
    return total


# revision 18
# speedup vs baseline: 1.1556x; 1.1556x over previous
"""Trainium2 Bass kernel for nn_NetCrossing (smoothed segment-crossing count).

Math restructure (vs the reference's per-pair s1..s4 formulation):
  For net with pins q_0..q_{P-1} and chain segments i (q_i -> q_{i+1}):
    G[i,p] = cross(d_i, q_p - q_i)   (= d1x_i*y_p - d1y_i*x_p - c1_i)
    s1(i,j)*s2(i,j) = G[i,j]*G[i,j+1] =: Q[i,j]
    s3(i,j)*s4(i,j) = Q[j,i]
  so with R = sigmoid(MU - Q):
    total = 0.5 * sum_{|i-j|>1, valid, same-side, masked} R[i,j]*R[j,i]
  The side weight w=(1+s_i*s_j)/2 in {0,1} and the |i-j|<=1 exclusion are
  folded into an additive pre-sigmoid kill tensor KU (host-precomputed):
  Q3 = Q - KU with KU = s_i*s_j*16384 - KILL, KILL in {16384, 32768};
  kept cells have KU == 0 (Q3 == Q exactly), excluded cells get
  Q3 >= ~16k so the sigmoid is exactly 0.

Sharding: nets are grouped by degree class (degree pattern tiles as
[2,3,4,5,6,8,10,12]; deg 2/3 nets have no non-adjacent segment pairs and are
dropped, masked nets are dropped) and distributed round-robin over 8 cores.
Per (core, class) buckets are padded to a fixed capacity with "kill" nets whose
pins sit on a huge convex polygon (every non-adjacent Q is hugely positive so
every sigmoid is exactly 0).

Device pipeline per class (DVE + ACT only — GpSimd measured 4-5x slower per
element, and extra engines inflate the Tile tail barrier):
  DVE: t1 = d1x (x) y_p ; t2 = d1y (x) x_p ; u = t1 - t2 ; G = u - c1
       Q = G[:,j]*G[:,j+1] ; Q3 = Q - KU
  ACT: R  = sigmoid(MU - Q3)            (natural layout)
       Rt = sigmoid(MU - Q3)            (transposed layout, second ACT op)
  DVE: T = R * Rt (unit stride) ; acc[:, ci] = sum(T)
"""

import math
import numpy as np

import concourse.bass as bass
import concourse.bacc as bacc
import concourse.mybir as mybir
from concourse import tile
from concourse.bass_utils import run_bass_kernel_spmd

F32 = mybir.dt.float32

MU = 0.01
LAMBDA = 1.0
CLASSES = [8, 10, 12, 4, 5, 6]
NCORES = 8
BIG = 16384.0
R0 = 1000.0                     # kill-polygon radius


def _kill_pattern(S):
    i = np.arange(S)
    k = np.full((S, S), BIG, np.float32)
    k[np.abs(i[:, None] - i[None, :]) <= 1] = 2.0 * BIG
    return k


def _pad_polygon(P):
    th = 2.0 * np.pi * np.arange(P) / P
    return (R0 * np.cos(th)).astype(np.float32), (R0 * np.sin(th)).astype(np.float32)


def _cls_cols(P, npp):
    S = P - 1
    # px, py [npp*P]; d1x, d1y, c1 [npp*S]; ku [npp*S*S]
    return npp * (2 * P + 3 * S + S * S)


def _layout(npps):
    cols = [_cls_cols(P, npp) for P, npp in zip(CLASSES, npps)]
    return cols, sum(cols)


def build_blobs(pos, flat_netpin, netpin_start, net_mask, pin_side):
    """Host-side shard/pack: FULL inputs -> per-core input blobs [128, COLS].

    Returns (blobs, npps): npps[i] = nets-per-partition for class i.
    """
    pos = np.asarray(pos)
    flat_netpin = np.asarray(flat_netpin).astype(np.int64)
    netpin_start = np.asarray(netpin_start).astype(np.int64)
    net_mask = np.asarray(net_mask).astype(bool)
    pin_side = np.asarray(pin_side)

    Ptot = pos.shape[0] // 2
    x = pos[:Ptot].astype(np.float32)
    y = pos[Ptot:].astype(np.float32)
    sidev = (2.0 * pin_side.astype(np.float32) - 1.0)

    deg = np.diff(netpin_start)

    per_class = []
    npps = []
    for P in CLASSES:
        S = P - 1
        nets = np.nonzero(net_mask & (deg == P))[0]
        starts = netpin_start[nets]
        pidx = starts[:, None] + np.arange(P)[None, :]
        pins = flat_netpin[pidx]
        per_class.append((x[pins], y[pins], sidev[pins[:, :S]]))
        worst = -(-len(nets) // NCORES)
        npps.append(max(1, -(-worst // 128)))

    cls_cols, COLS = _layout(npps)
    blobs = [np.empty((128, COLS), np.float32) for _ in range(NCORES)]

    col = 0
    for ci, P in enumerate(CLASSES):
        S = P - 1
        npp = npps[ci]
        cap = 128 * npp
        pxc, pyc, spc = per_class[ci]
        padx, pady = _pad_polygon(P)
        killp = _kill_pattern(S)

        for core in range(NCORES):
            mpx = pxc[core::NCORES]
            m = mpx.shape[0]
            if m > cap:
                raise RuntimeError(
                    f"class deg={P} core={core}: {m} nets exceeds capacity {cap}"
                )
            bx = np.broadcast_to(padx, (cap, P)).copy()
            by = np.broadcast_to(pady, (cap, P)).copy()
            bs = np.ones((cap, S), np.float32)
            bx[:m] = mpx
            by[:m] = pyc[core::NCORES]
            bs[:m] = spc[core::NCORES]

            d1x = bx[:, 1:] - bx[:, :-1]
            d1y = by[:, 1:] - by[:, :-1]
            c1 = d1x * by[:, :S] - d1y * bx[:, :S]
            ku = (BIG * bs[:, :, None] * bs[:, None, :]) - killp[None, :, :]

            b = blobs[core]
            c = col
            for arr, w in ((bx, P), (by, P), (d1x, S), (d1y, S), (c1, S),
                           (ku.reshape(cap, S * S), S * S)):
                b[:, c:c + npp * w] = arr.reshape(128, npp * w)
                c += npp * w
        col += cls_cols[ci]

    return blobs, npps


def _emit_program(npps):
    """Build the Bass/Tile program (shared by all 8 cores, SPMD)."""
    cls_cols, COLS = _layout(npps)

    nc = bacc.Bacc()
    blob = nc.declare_dram_parameter("blob", [128, COLS], F32, isOutput=False)
    outp = nc.declare_dram_parameter("out", [128, 1], F32, isOutput=True)

    AX = mybir.AxisListType
    OP = mybir.AluOpType
    ACTF = mybir.ActivationFunctionType

    with tile.TileContext(nc) as tc:
        with (
            tc.tile_pool(name="io", bufs=1) as io,
            tc.tile_pool(name="work", bufs=2) as work,
        ):
            cls_in = []
            col = 0
            for ci, P in enumerate(CLASSES):
                t = io.tile([128, cls_cols[ci]], F32, tag=f"in_{ci}")
                nc.sync.dma_start(t[:], blob[:, col:col + cls_cols[ci]])
                col += cls_cols[ci]
                cls_in.append(t)

            acc = io.tile([128, len(CLASSES)], F32)
            mu_t = io.tile([128, 1], F32)
            nc.vector.memset(mu_t[:], MU)

            for ci, P in enumerate(CLASSES):
                S = P - 1
                npp = npps[ci]

                sb = cls_in[ci][:]
                c = 0

                def take(w, ndims):
                    nonlocal c
                    v = sb[:, c:c + npp * w]
                    c += npp * w
                    if ndims == 3:
                        return v.rearrange("p (n q) -> p n q", n=npp)
                    return v.rearrange("p (n i j) -> p n i j", n=npp, i=S)

                px = take(P, 3)
                py = take(P, 3)
                d1x = take(S, 3)
                d1y = take(S, 3)
                c1 = take(S, 3)
                ku4 = take(S * S, 4)

                def t4(name, a, b):
                    t = work.tile([128, npp * a * b], F32, tag=name)
                    return t[:].rearrange("p (n i j) -> p n i j", n=npp, i=a)

                # G stage: G[i,p] = d1x_i*y_p - d1y_i*x_p - c1_i
                sh4 = [128, npp, S, P]
                t1 = t4("t1", S, P)
                t2 = t4("t2", S, P)
                u4 = t4("u4", S, P)
                g4 = t4("g4", S, P)
                nc.vector.tensor_mul(
                    t1, d1x.unsqueeze(3).broadcast_to(sh4),
                    py.unsqueeze(2).broadcast_to(sh4),
                )
                nc.vector.tensor_mul(
                    t2, d1y.unsqueeze(3).broadcast_to(sh4),
                    px.unsqueeze(2).broadcast_to(sh4),
                )
                nc.vector.tensor_sub(u4, t1, t2)
                nc.vector.tensor_sub(g4, u4, c1.unsqueeze(3).broadcast_to(sh4))

                # Q = G[:, :, i, j] * G[:, :, i, j+1];  Q3 = Q - KU
                q4 = t4("q4", S, S)
                q3 = t4("q3", S, S)
                nc.vector.tensor_mul(q4, g4[:, :, :, 0:S], g4[:, :, :, 1:P])
                nc.vector.tensor_sub(q3, q4, ku4)

                # R = sigmoid(MU - Q3), emitted twice: natural + transposed
                # (distinct slots per class: the ACT write must carry only one
                # HW wait, so it cannot afford a WAR wait from slot reuse)
                r4 = t4(f"r4_{ci}", S, S)
                rt4 = t4(f"rt4_{ci}", S, S)
                q3f = q3.rearrange("p n i j -> p (n i j)")
                nc.scalar.activation(
                    r4.rearrange("p n i j -> p (n i j)"), q3f,
                    ACTF.Sigmoid, bias=mu_t[:], scale=-1.0,
                )
                nc.scalar.activation(
                    rt4.transpose([0, 1, 3, 2]), q3,
                    ACTF.Sigmoid, bias=mu_t[:], scale=-1.0,
                )

                # T = R * Rt (pure unit stride); acc[:, ci] = sum(T)
                ts4 = t4("ts4", S, S)
                nc.vector.tensor_mul(
                    ts4.rearrange("p n i j -> p (n i j)"),
                    r4.rearrange("p n i j -> p (n i j)"),
                    rt4.rearrange("p n i j -> p (n i j)"),
                )
                nc.vector.tensor_reduce(
                    acc[:, ci:ci + 1],
                    ts4.rearrange("p n i j -> p (n i j)"),
                    AX.X, OP.add,
                )

            accfin = io.tile([128, 1], F32)
            nc.vector.tensor_reduce(accfin[:], acc[:], AX.X, OP.add)
            nc.sync.dma_start(outp[:], accfin[:])

    # bacc legalization (splits multi-sem waits: HW allows 1 wait/instruction)
    nc.compile()
    return nc


def run_on_hw(blobs, npps, trace=False, **kw):
    nc = _emit_program(npps)
    in_maps = [{"blob": blobs[c]} for c in range(NCORES)]
    br = run_bass_kernel_spmd(nc, in_maps, list(range(NCORES)), trace=trace, **kw)
    total = 0.0
    for c in range(NCORES):
        total += float(np.asarray(br.results[c]["out"], np.float64).sum())
    total *= 0.5 * LAMBDA
    return np.float32(total), br


def kernel(pos, flat_netpin, netpin_start, net_mask, pin_side):
    blobs, npps = build_blobs(pos, flat_netpin, netpin_start, net_mask, pin_side)
    total, _ = run_on_hw(blobs, npps, trace=False)
    return total


# revision 23
# speedup vs baseline: 1.1562x; 1.0005x over previous
"""Trainium2 Bass kernel for nn_NetCrossing (smoothed segment-crossing count).

Math restructure (vs the reference's per-pair s1..s4 formulation):
  For net with pins q_0..q_{P-1} and chain segments i (q_i -> q_{i+1}):
    G[i,p] = cross(d_i, q_p - q_i)   (= d1x_i*y_p - d1y_i*x_p - c1_i)
    s1(i,j)*s2(i,j) = G[i,j]*G[i,j+1] =: Q[i,j]
    s3(i,j)*s4(i,j) = Q[j,i]
  so with R = sigmoid(MU - Q):
    total = 0.5 * sum_{|i-j|>1, valid, same-side, masked} R[i,j]*R[j,i]
  The side weight w=(1+s_i*s_j)/2 in {0,1} and the |i-j|<=1 exclusion are
  folded into an additive pre-sigmoid kill tensor KU (host-precomputed):
  Q3 = Q - KU, KU = s_i*s_j*16384 - KILL, KILL in {16384, 32768}; kept cells
  have KU == 0 (Q3 == Q exactly), excluded cells get Q3 >= ~16k so the
  sigmoid is exactly 0.

Sharding: nets are grouped by degree class (degree pattern tiles as
[2,3,4,5,6,8,10,12]; deg 2/3 nets have no non-adjacent segment pairs and are
dropped, masked nets are dropped) and distributed round-robin over 8 cores.
Per (core, class) buckets are padded to a fixed capacity with "kill" nets whose
pins sit on a huge convex polygon (every non-adjacent Q is hugely positive so
every sigmoid is exactly 0).

Raw Bacc implementation (no TileContext): the Tile kernel-tail EVSEM barrier
costs ~17us, which dominates a ~30us kernel. Hand-placed semaphores instead:
  SYNC:   per-class input DMA -> sbuf; final accfin -> out DMA
  VECTOR: per class: t1 = d1x(x)y, t2 = d1y(x)x, u = t1-t2, G = u-c1,
          Q = G_j*G_{j+1}, Q3 = Q-KU (inc s_q3); lagged by 2 classes:
          T = R*Rt (unit stride), acc[:,ci] = sum(T)
  SCALAR: per class: R = sigmoid(MU - Q3) natural + transposed (inc s_act)
The 2-class lag lets ACT's sigmoids finish before DVE needs them, with no
scratch-reuse hazards (per-class q3/r/rt buffers).
"""

import math
import numpy as np

import concourse.bass as bass
import concourse.bacc as bacc
import concourse.mybir as mybir
from concourse.bass_utils import run_bass_kernel_spmd

F32 = mybir.dt.float32

MU = 0.01
LAMBDA = 1.0
CLASSES = [8, 10, 12, 4, 5, 6]
NCORES = 8
BIG = 16384.0
R0 = 1000.0                     # kill-polygon radius


def _kill_pattern(S):
    i = np.arange(S)
    k = np.full((S, S), BIG, np.float32)
    k[np.abs(i[:, None] - i[None, :]) <= 1] = 2.0 * BIG
    return k


def _pad_polygon(P):
    th = 2.0 * np.pi * np.arange(P) / P
    return (R0 * np.cos(th)).astype(np.float32), (R0 * np.sin(th)).astype(np.float32)


def _cls_cols(P, npp):
    S = P - 1
    # px, py [npp*P]; d1x, d1y, c1 [npp*S]; ku [npp*S*S]
    return npp * (2 * P + 3 * S + S * S)


def _layout(npps):
    cols = [_cls_cols(P, npp) for P, npp in zip(CLASSES, npps)]
    cols[0] += 1                 # trailing MU bias column in class-0 chunk
    return cols, sum(cols)


def build_blobs(pos, flat_netpin, netpin_start, net_mask, pin_side):
    """Host-side shard/pack: FULL inputs -> per-core input blobs [128, COLS].

    Returns (blobs, npps): npps[i] = nets-per-partition for class i.
    """
    pos = np.asarray(pos)
    flat_netpin = np.asarray(flat_netpin).astype(np.int64)
    netpin_start = np.asarray(netpin_start).astype(np.int64)
    net_mask = np.asarray(net_mask).astype(bool)
    pin_side = np.asarray(pin_side)

    Ptot = pos.shape[0] // 2
    x = pos[:Ptot].astype(np.float32)
    y = pos[Ptot:].astype(np.float32)
    sidev = (2.0 * pin_side.astype(np.float32) - 1.0)

    deg = np.diff(netpin_start)

    per_class = []
    npps = []
    for P in CLASSES:
        S = P - 1
        nets = np.nonzero(net_mask & (deg == P))[0]
        starts = netpin_start[nets]
        pidx = starts[:, None] + np.arange(P)[None, :]
        pins = flat_netpin[pidx]
        per_class.append((x[pins], y[pins], sidev[pins[:, :S]]))
        worst = -(-len(nets) // NCORES)
        npps.append(max(1, -(-worst // 128)))

    cls_cols, COLS = _layout(npps)
    blobs = [np.empty((128, COLS), np.float32) for _ in range(NCORES)]

    col = 0
    for ci, P in enumerate(CLASSES):
        S = P - 1
        npp = npps[ci]
        cap = 128 * npp
        pxc, pyc, spc = per_class[ci]
        padx, pady = _pad_polygon(P)
        killp = _kill_pattern(S)

        for core in range(NCORES):
            mpx = pxc[core::NCORES]
            m = mpx.shape[0]
            if m > cap:
                raise RuntimeError(
                    f"class deg={P} core={core}: {m} nets exceeds capacity {cap}"
                )
            bx = np.broadcast_to(padx, (cap, P)).copy()
            by = np.broadcast_to(pady, (cap, P)).copy()
            bs = np.ones((cap, S), np.float32)
            bx[:m] = mpx
            by[:m] = pyc[core::NCORES]
            bs[:m] = spc[core::NCORES]

            d1x = bx[:, 1:] - bx[:, :-1]
            d1y = by[:, 1:] - by[:, :-1]
            c1 = d1x * by[:, :S] - d1y * bx[:, :S]
            ku = (BIG * bs[:, :, None] * bs[:, None, :]) - killp[None, :, :]

            b = blobs[core]
            c = col
            for arr, w in ((bx, P), (by, P), (d1x, S), (d1y, S), (c1, S),
                           (ku.reshape(cap, S * S), S * S)):
                b[:, c:c + npp * w] = arr.reshape(128, npp * w)
                c += npp * w
            if ci == 0:
                b[:, c] = MU
        col += cls_cols[ci]

    return blobs, npps


def _emit_program(npps):
    """Build the raw Bacc program (shared by all 8 cores, SPMD)."""
    cls_cols, COLS = _layout(npps)
    NCLS = len(CLASSES)

    nc = bacc.Bacc()
    blob = nc.declare_dram_parameter("blob", [128, COLS], F32, isOutput=False)
    outp = nc.declare_dram_parameter("out", [128, 1], F32, isOutput=True)

    AX = mybir.AxisListType
    OP = mybir.AluOpType
    ACTF = mybir.ActivationFunctionType

    # SBUF allocations
    in_t = [nc.alloc_sbuf_tensor(f"in_{ci}", [128, cls_cols[ci]], F32)
            for ci in range(NCLS)]
    maxSP = max(npps[ci] * (P - 1) * P for ci, P in enumerate(CLASSES))
    t1 = nc.alloc_sbuf_tensor("t1", [128, maxSP], F32)
    t2 = nc.alloc_sbuf_tensor("t2", [128, maxSP], F32)
    u4 = nc.alloc_sbuf_tensor("u4", [128, maxSP], F32)
    g4 = nc.alloc_sbuf_tensor("g4", [128, maxSP], F32)
    maxC = max(npps[ci] * (P - 1) * (P - 1) for ci, P in enumerate(CLASSES))
    q4 = nc.alloc_sbuf_tensor("q4", [128, maxC], F32)
    ts = nc.alloc_sbuf_tensor("ts", [128, maxC], F32)
    q3 = [nc.alloc_sbuf_tensor(f"q3_{ci}", [128, npps[ci] * (P - 1) ** 2], F32)
          for ci, P in enumerate(CLASSES)]
    r_t = [nc.alloc_sbuf_tensor(f"r_{ci}", [128, npps[ci] * (P - 1) ** 2], F32)
           for ci, P in enumerate(CLASSES)]
    rt_t = [nc.alloc_sbuf_tensor(f"rt_{ci}", [128, npps[ci] * (P - 1) ** 2], F32)
            for ci, P in enumerate(CLASSES)]
    acc = nc.alloc_sbuf_tensor("acc", [128, NCLS], F32)
    accfin = nc.alloc_sbuf_tensor("accfin", [128, 1], F32)

    def views(ci):
        P = CLASSES[ci]
        S = P - 1
        npp = npps[ci]
        sb = in_t[ci][:]
        c = 0
        out = []
        for w in (P, P, S, S, S):
            out.append(sb[:, c:c + npp * w].rearrange("p (n q) -> p n q", n=npp))
            c += npp * w
        out.append(sb[:, c:c + npp * S * S]
                   .rearrange("p (n i j) -> p n i j", n=npp, i=S))
        return out

    def r4(th, ci, a, b):
        npp = npps[ci]
        return th[:, :npp * a * b].rearrange("p (n i j) -> p n i j", n=npp, i=a)

    mu_ap = in_t[0][:, cls_cols[0] - 1:cls_cols[0]]

    import contextlib
    with contextlib.ExitStack() as stack:
        # per-class DMA sems: SWDGE queues complete out of order, so one
        # shared counting sem cannot tell which class's data landed
        dma_in = [stack.enter_context(nc.semaphore(f"dma_in{ci}"))
                  for ci in range(NCLS)]
        s_q3 = stack.enter_context(nc.semaphore("s_q3"))
        s_act = stack.enter_context(nc.semaphore("s_act"))
        s_fin = stack.enter_context(nc.semaphore("s_fin"))
        dma_out = stack.enter_context(nc.semaphore("dma_out"))
        block = stack.enter_context(nc.Block())

        @block.gpsimd
        def _(gpsimd):
            col = 0
            for ci in range(NCLS):
                nc.gpsimd.dma_start(
                    in_t[ci][:], blob[:, col:col + cls_cols[ci]]
                ).then_inc(dma_in[ci], 16)
                col += cls_cols[ci]

        @block.sync
        def _(sync):
            nc.sync.wait_ge(s_fin, 1)
            nc.sync.dma_start(outp[:], accfin[:]).then_inc(dma_out, 16)
            nc.sync.wait_ge(dma_out, 16)

        @block.vector
        def _(vector):
            # explicit drains mark same-engine RAW/WAR points (the DVE pipe
            # auto-flushes per op on HW; the drain is ~free and satisfies the
            # race checker's sync-with-drain pattern)
            def emit_T(ci):
                # T = R * Rt (unit stride); acc[:, ci] = sum(T)
                P = CLASSES[ci]
                S = P - 1
                n = npps[ci] * S * S
                nc.vector.wait_ge(s_act, 2 * (ci + 1))
                nc.vector.drain()
                nc.vector.tensor_mul(ts[:, :n], r_t[ci][:], rt_t[ci][:])
                nc.vector.drain()
                nc.vector.tensor_reduce(
                    acc[:, ci:ci + 1], ts[:, :n], AX.X, OP.add)

            for ci in range(NCLS):
                P = CLASSES[ci]
                S = P - 1
                npp = npps[ci]
                nc.vector.wait_ge(dma_in[ci], 16)
                if ci >= 2:
                    emit_T(ci - 2)
                    nc.vector.drain()
                px, py, d1x, d1y, c1, ku4 = views(ci)
                sh4 = [128, npp, S, P]
                t1v = r4(t1, ci, S, P)
                t2v = r4(t2, ci, S, P)
                u4v = r4(u4, ci, S, P)
                g4v = r4(g4, ci, S, P)
                nc.vector.tensor_mul(
                    t1v, d1x.unsqueeze(3).broadcast_to(sh4),
                    py.unsqueeze(2).broadcast_to(sh4))
                nc.vector.tensor_mul(
                    t2v, d1y.unsqueeze(3).broadcast_to(sh4),
                    px.unsqueeze(2).broadcast_to(sh4))
                nc.vector.drain()
                nc.vector.tensor_sub(u4v, t1v, t2v)
                nc.vector.drain()
                nc.vector.tensor_sub(g4v, u4v, c1.unsqueeze(3).broadcast_to(sh4))
                nc.vector.drain()
                q4v = r4(q4, ci, S, S)
                nc.vector.tensor_mul(q4v, g4v[:, :, :, 0:S], g4v[:, :, :, 1:P])
                nc.vector.drain()
                nc.vector.tensor_sub(
                    r4(q3[ci][:], ci, S, S), q4v, ku4).then_inc(s_q3, 1)

            emit_T(NCLS - 2)
            nc.vector.drain()
            emit_T(NCLS - 1)
            nc.vector.drain()
            nc.vector.tensor_reduce(
                accfin[:], acc[:], AX.X, OP.add).then_inc(s_fin, 1)

        @block.scalar
        def _(scalar):
            for ci in range(NCLS):
                nc.scalar.wait_ge(s_q3, ci + 1)
                q3f = q3[ci][:]
                nc.scalar.activation(
                    r_t[ci][:], q3f, ACTF.Sigmoid, bias=mu_ap, scale=-1.0)
                P = CLASSES[ci]
                S = P - 1
                nc.scalar.activation(
                    r4(rt_t[ci][:], ci, S, S).transpose([0, 1, 3, 2]),
                    r4(q3f, ci, S, S),
                    ACTF.Sigmoid, bias=mu_ap, scale=-1.0,
                ).then_inc(s_act, 2)

    # bacc legalization (splits multi-sem waits: HW allows 1 wait/instruction)
    nc.compile()
    return nc


def run_on_hw(blobs, npps, trace=False, **kw):
    nc = _emit_program(npps)
    in_maps = [{"blob": blobs[c]} for c in range(NCORES)]
    br = run_bass_kernel_spmd(nc, in_maps, list(range(NCORES)), trace=trace, **kw)
    total = 0.0
    for c in range(NCORES):
        total += float(np.asarray(br.results[c]["out"], np.float64).sum())
    total *= 0.5 * LAMBDA
    return np.float32(total), br


def kernel(pos, flat_netpin, netpin_start, net_mask, pin_side):
    blobs, npps = build_blobs(pos, flat_netpin, netpin_start, net_mask, pin_side)
    total, _ = run_on_hw(blobs, npps, trace=False)
    return total


# revision 25
# speedup vs baseline: 1.1705x; 1.0124x over previous
"""Trainium2 Bass kernel for nn_NetCrossing (smoothed segment-crossing count).

Math restructure (vs the reference's per-pair s1..s4 formulation):
  For net with pins q_0..q_{P-1} and chain segments i (q_i -> q_{i+1}):
    G[i,p] = cross(d_i, q_p - q_i)   (= d1x_i*y_p - d1y_i*x_p - c1_i)
    s1(i,j)*s2(i,j) = G[i,j]*G[i,j+1] =: Q[i,j]
    s3(i,j)*s4(i,j) = Q[j,i]
  so with R = sigmoid(MU - Q):
    total = 0.5 * sum_{|i-j|>1, valid, same-side, masked} R[i,j]*R[j,i]
  The side weight w=(1+s_i*s_j)/2 in {0,1} and the |i-j|<=1 exclusion are
  folded into an additive pre-sigmoid kill tensor KU (host-precomputed):
  Q3 = Q - KU, KU = s_i*s_j*16384 - KILL, KILL in {16384, 32768}; kept cells
  have KU == 0 (Q3 == Q exactly), excluded cells get Q3 >= ~16k so the
  sigmoid is exactly 0.

Sharding: nets are grouped by degree class (degree pattern tiles as
[2,3,4,5,6,8,10,12]; deg 2/3 nets have no non-adjacent segment pairs and are
dropped, masked nets are dropped) and distributed round-robin over 8 cores.
Per (core, class) buckets are padded to a fixed capacity with "kill" nets whose
pins sit on a huge convex polygon (every non-adjacent Q is hugely positive so
every sigmoid is exactly 0).

Raw Bacc implementation (no TileContext): the Tile kernel-tail EVSEM barrier
costs ~17us, which dominates a ~30us kernel. Hand-placed semaphores instead:
  SYNC:   per-class input DMA -> sbuf; final accfin -> out DMA
  VECTOR: per class: t1 = d1x(x)y, t2 = d1y(x)x, u = t1-t2, G = u-c1,
          Q = G_j*G_{j+1}, Q3 = Q-KU (inc s_q3); lagged by 2 classes:
          T = R*Rt (unit stride), acc[:,ci] = sum(T)
  SCALAR: per class: R = sigmoid(MU - Q3) natural + transposed (inc s_act)
The 2-class lag lets ACT's sigmoids finish before DVE needs them, with no
scratch-reuse hazards (per-class q3/r/rt buffers).
"""

import math
import numpy as np

import concourse.bass as bass
import concourse.bacc as bacc
import concourse.mybir as mybir
from concourse.bass_utils import run_bass_kernel_spmd

F32 = mybir.dt.float32

MU = 0.01
LAMBDA = 1.0
CLASSES = [4, 5, 6, 8, 10, 12]
NCORES = 8
BIG = 16384.0
R0 = 1000.0                     # kill-polygon radius


def _kill_pattern(S):
    i = np.arange(S)
    k = np.full((S, S), BIG, np.float32)
    k[np.abs(i[:, None] - i[None, :]) <= 1] = 2.0 * BIG
    return k


def _pad_polygon(P):
    th = 2.0 * np.pi * np.arange(P) / P
    return (R0 * np.cos(th)).astype(np.float32), (R0 * np.sin(th)).astype(np.float32)


def _cls_cols(P, npp):
    S = P - 1
    # px, py [npp*P]; d1x, d1y, c1 [npp*S]; ku [npp*S*S]
    return npp * (2 * P + 3 * S + S * S)


def _layout(npps):
    cols = [_cls_cols(P, npp) for P, npp in zip(CLASSES, npps)]
    cols[0] += 1                 # trailing MU bias column in class-0 chunk
    return cols, sum(cols)


def build_blobs(pos, flat_netpin, netpin_start, net_mask, pin_side):
    """Host-side shard/pack: FULL inputs -> per-core input blobs [128, COLS].

    Returns (blobs, npps): npps[i] = nets-per-partition for class i.
    """
    pos = np.asarray(pos)
    flat_netpin = np.asarray(flat_netpin).astype(np.int64)
    netpin_start = np.asarray(netpin_start).astype(np.int64)
    net_mask = np.asarray(net_mask).astype(bool)
    pin_side = np.asarray(pin_side)

    Ptot = pos.shape[0] // 2
    x = pos[:Ptot].astype(np.float32)
    y = pos[Ptot:].astype(np.float32)
    sidev = (2.0 * pin_side.astype(np.float32) - 1.0)

    deg = np.diff(netpin_start)

    per_class = []
    npps = []
    for P in CLASSES:
        S = P - 1
        nets = np.nonzero(net_mask & (deg == P))[0]
        starts = netpin_start[nets]
        pidx = starts[:, None] + np.arange(P)[None, :]
        pins = flat_netpin[pidx]
        per_class.append((x[pins], y[pins], sidev[pins[:, :S]]))
        worst = -(-len(nets) // NCORES)
        npps.append(max(1, -(-worst // 128)))

    cls_cols, COLS = _layout(npps)
    blobs = [np.empty((128, COLS), np.float32) for _ in range(NCORES)]

    col = 0
    for ci, P in enumerate(CLASSES):
        S = P - 1
        npp = npps[ci]
        cap = 128 * npp
        pxc, pyc, spc = per_class[ci]
        padx, pady = _pad_polygon(P)
        killp = _kill_pattern(S)

        for core in range(NCORES):
            mpx = pxc[core::NCORES]
            m = mpx.shape[0]
            if m > cap:
                raise RuntimeError(
                    f"class deg={P} core={core}: {m} nets exceeds capacity {cap}"
                )
            bx = np.broadcast_to(padx, (cap, P)).copy()
            by = np.broadcast_to(pady, (cap, P)).copy()
            bs = np.ones((cap, S), np.float32)
            bx[:m] = mpx
            by[:m] = pyc[core::NCORES]
            bs[:m] = spc[core::NCORES]

            d1x = bx[:, 1:] - bx[:, :-1]
            d1y = by[:, 1:] - by[:, :-1]
            c1 = d1x * by[:, :S] - d1y * bx[:, :S]
            ku = (BIG * bs[:, :, None] * bs[:, None, :]) - killp[None, :, :]

            b = blobs[core]
            c = col
            for arr, w in ((bx, P), (by, P), (d1x, S), (d1y, S), (c1, S),
                           (ku.reshape(cap, S * S), S * S)):
                b[:, c:c + npp * w] = arr.reshape(128, npp * w)
                c += npp * w
            if ci == 0:
                b[:, c] = MU
        col += cls_cols[ci]

    return blobs, npps


def _emit_program(npps):
    """Build the raw Bacc program (shared by all 8 cores, SPMD)."""
    cls_cols, COLS = _layout(npps)
    NCLS = len(CLASSES)

    nc = bacc.Bacc()
    blob = nc.declare_dram_parameter("blob", [128, COLS], F32, isOutput=False)
    outp = nc.declare_dram_parameter("out", [128, 1], F32, isOutput=True)

    AX = mybir.AxisListType
    OP = mybir.AluOpType
    ACTF = mybir.ActivationFunctionType

    # SBUF allocations
    in_t = [nc.alloc_sbuf_tensor(f"in_{ci}", [128, cls_cols[ci]], F32)
            for ci in range(NCLS)]
    maxSP = max(npps[ci] * (P - 1) * P for ci, P in enumerate(CLASSES))
    t1 = nc.alloc_sbuf_tensor("t1", [128, maxSP], F32)
    t2 = nc.alloc_sbuf_tensor("t2", [128, maxSP], F32)
    u4 = nc.alloc_sbuf_tensor("u4", [128, maxSP], F32)
    g4 = nc.alloc_sbuf_tensor("g4", [128, maxSP], F32)
    maxC = max(npps[ci] * (P - 1) * (P - 1) for ci, P in enumerate(CLASSES))
    q4 = nc.alloc_sbuf_tensor("q4", [128, maxC], F32)
    ts = nc.alloc_sbuf_tensor("ts", [128, maxC], F32)
    q3 = [nc.alloc_sbuf_tensor(f"q3_{ci}", [128, npps[ci] * (P - 1) ** 2], F32)
          for ci, P in enumerate(CLASSES)]
    r_t = [nc.alloc_sbuf_tensor(f"r_{ci}", [128, npps[ci] * (P - 1) ** 2], F32)
           for ci, P in enumerate(CLASSES)]
    rt_t = [nc.alloc_sbuf_tensor(f"rt_{ci}", [128, npps[ci] * (P - 1) ** 2], F32)
            for ci, P in enumerate(CLASSES)]
    acc = nc.alloc_sbuf_tensor("acc", [128, NCLS], F32)
    accfin = nc.alloc_sbuf_tensor("accfin", [128, 1], F32)

    def views(ci):
        P = CLASSES[ci]
        S = P - 1
        npp = npps[ci]
        sb = in_t[ci][:]
        c = 0
        out = []
        for w in (P, P, S, S, S):
            out.append(sb[:, c:c + npp * w].rearrange("p (n q) -> p n q", n=npp))
            c += npp * w
        out.append(sb[:, c:c + npp * S * S]
                   .rearrange("p (n i j) -> p n i j", n=npp, i=S))
        return out

    def r4(th, ci, a, b):
        npp = npps[ci]
        return th[:, :npp * a * b].rearrange("p (n i j) -> p n i j", n=npp, i=a)

    mu_ap = in_t[0][:, cls_cols[0] - 1:cls_cols[0]]

    import contextlib
    with contextlib.ExitStack() as stack:
        # per-class DMA sems: SWDGE queues complete out of order, so one
        # shared counting sem cannot tell which class's data landed
        dma_in = [stack.enter_context(nc.semaphore(f"dma_in{ci}"))
                  for ci in range(NCLS)]
        s_q3 = stack.enter_context(nc.semaphore("s_q3"))
        s_act = stack.enter_context(nc.semaphore("s_act"))
        s_fin = stack.enter_context(nc.semaphore("s_fin"))
        dma_out = stack.enter_context(nc.semaphore("dma_out"))
        # no_gpsimd_drain: skip the ~6.5us SWDGE dge_drain at block exit and
        # use the sequencer-only (no EVSEM butterfly) end barrier
        block = stack.enter_context(nc.Block(no_gpsimd_drain=True))

        @block.gpsimd
        def _(gpsimd):
            col = 0
            for ci in range(NCLS):
                nc.gpsimd.dma_start(
                    in_t[ci][:], blob[:, col:col + cls_cols[ci]]
                ).then_inc(dma_in[ci], 16)
                col += cls_cols[ci]

        @block.sync
        def _(sync):
            nc.sync.wait_ge(s_fin, 1)
            nc.sync.dma_start(outp[:], accfin[:]).then_inc(dma_out, 16)
            nc.sync.wait_ge(dma_out, 16)

        @block.vector
        def _(vector):
            # explicit drains mark same-engine RAW/WAR points (the DVE pipe
            # auto-flushes per op on HW; the drain is ~free and satisfies the
            # race checker's sync-with-drain pattern)
            def emit_T(ci):
                # T = R * Rt (unit stride); acc[:, ci] = sum(T)
                P = CLASSES[ci]
                S = P - 1
                n = npps[ci] * S * S
                nc.vector.wait_ge(s_act, 2 * (ci + 1))
                nc.vector.drain()
                nc.vector.tensor_mul(ts[:, :n], r_t[ci][:], rt_t[ci][:])
                nc.vector.drain()
                nc.vector.tensor_reduce(
                    acc[:, ci:ci + 1], ts[:, :n], AX.X, OP.add)

            for ci in range(NCLS):
                P = CLASSES[ci]
                S = P - 1
                npp = npps[ci]
                nc.vector.wait_ge(dma_in[ci], 16)
                if ci >= 2:
                    emit_T(ci - 2)
                    nc.vector.drain()
                px, py, d1x, d1y, c1, ku4 = views(ci)
                sh4 = [128, npp, S, P]
                t1v = r4(t1, ci, S, P)
                t2v = r4(t2, ci, S, P)
                u4v = r4(u4, ci, S, P)
                g4v = r4(g4, ci, S, P)
                nc.vector.tensor_mul(
                    t1v, d1x.unsqueeze(3).broadcast_to(sh4),
                    py.unsqueeze(2).broadcast_to(sh4))
                nc.vector.tensor_mul(
                    t2v, d1y.unsqueeze(3).broadcast_to(sh4),
                    px.unsqueeze(2).broadcast_to(sh4))
                nc.vector.drain()
                nc.vector.tensor_sub(u4v, t1v, t2v)
                nc.vector.drain()
                nc.vector.tensor_sub(g4v, u4v, c1.unsqueeze(3).broadcast_to(sh4))
                nc.vector.drain()
                q4v = r4(q4, ci, S, S)
                nc.vector.tensor_mul(q4v, g4v[:, :, :, 0:S], g4v[:, :, :, 1:P])
                nc.vector.drain()
                nc.vector.tensor_sub(
                    r4(q3[ci][:], ci, S, S), q4v, ku4).then_inc(s_q3, 1)

            emit_T(NCLS - 2)
            nc.vector.drain()
            emit_T(NCLS - 1)
            nc.vector.drain()
            nc.vector.tensor_reduce(
                accfin[:], acc[:], AX.X, OP.add).then_inc(s_fin, 1)

        @block.scalar
        def _(scalar):
            for ci in range(NCLS):
                nc.scalar.wait_ge(s_q3, ci + 1)
                q3f = q3[ci][:]
                nc.scalar.activation(
                    r_t[ci][:], q3f, ACTF.Sigmoid, bias=mu_ap, scale=-1.0)
                P = CLASSES[ci]
                S = P - 1
                nc.scalar.activation(
                    r4(rt_t[ci][:], ci, S, S).transpose([0, 1, 3, 2]),
                    r4(q3f, ci, S, S),
                    ACTF.Sigmoid, bias=mu_ap, scale=-1.0,
                ).then_inc(s_act, 2)

    # bacc legalization (splits multi-sem waits: HW allows 1 wait/instruction)
    nc.compile()
    return nc


def run_on_hw(blobs, npps, trace=False, **kw):
    nc = _emit_program(npps)
    in_maps = [{"blob": blobs[c]} for c in range(NCORES)]
    br = run_bass_kernel_spmd(nc, in_maps, list(range(NCORES)), trace=trace, **kw)
    total = 0.0
    for c in range(NCORES):
        total += float(np.asarray(br.results[c]["out"], np.float64).sum())
    total *= 0.5 * LAMBDA
    return np.float32(total), br


def kernel(pos, flat_netpin, netpin_start, net_mask, pin_side):
    blobs, npps = build_blobs(pos, flat_netpin, netpin_start, net_mask, pin_side)
    total, _ = run_on_hw(blobs, npps, trace=False)
    return total
